# revision 14
# baseline (speedup 1.0000x reference)
"""Trainium2 Bass kernel for nn_DenseOnUp (gnn_message_passing).

Math: reference computes
    up = elu_mlp(x[sel])            # gather
    out = x + scatter_add(sel, up)  # scatter back to the SAME indices
Since scatter indices equal gather indices, duplicate selections
contribute identical MLP values, so
    out[n] = x[n] + cnt(n) * elu_mlp(x[n])   with cnt(n) = selection count.

Strategy (8 cores, data-parallel over rows): fully DENSE streaming.
Indirect (per-row) DMAs cost ~36-75 ns/row on HW (descriptor bound), so a
gather/scatter formulation has a ~1.3 ms/core floor. Instead every row is
pushed through the MLP (PE has headroom) and multiplied by its count
(zero for unselected rows), which keeps all DMA traffic dense/contiguous:

  per 512-row block:
    load x  (1 HWDGE dma per 1024 rows, sync queue)
    PE transpose -> feature-major; MLP in bf16 with the elu identity
        elu(z)+1 = min(exp(z), z+1)
    biases are folded into PSUM via rank-1 (K=1) matmuls so the exp's
        bias is the constant -1 and elementwise ops fuse across the
        two 128-feature output tiles; exp on ACT, min on DVE/Pool
    PE transpose back; y = x + cnt*h' - cnt on DVE
    store y (SWDGE dma, gpsimd queue, keeps HWDGE free for loads)

Precision: bf16 matmul operands / f32 PSUM; rows with cnt=0 produce
exactly x (0*h' - 0 + x). rel err vs f32 reference ~1e-3..1e-2 << 2e-2.
"""

import numpy as np

import concourse.bacc as bacc
import concourse.mybir as mybir
import concourse.tile as tile
from concourse.bass_utils import run_bass_kernel_spmd
from concourse.masks import make_identity

F32 = mybir.dt.float32
BF16 = mybir.dt.bfloat16
AF = mybir.ActivationFunctionType
OP = mybir.AluOpType

N_CORES = 8
N = 400000
F = 128
H = 256
RPC = N // N_CORES          # rows per core = 50000
BLK = 512                   # rows per MLP block
LOAD = 1024                 # rows per input DMA (2 blocks)
NL = 49                     # loads per core (last one overlaps)
NB = 98                     # blocks per core
JPB = BLK // 128            # 128-row sub-tiles per block = 4

LAST_RUN_RESULTS = None
LAST_NC = None
LAST_IN_MAPS = None


def _load_starts():
    s = [min(i * LOAD, RPC - LOAD) for i in range(NL)]
    return s


def _block_starts():
    ls = _load_starts()
    return [ls[b // 2] + (b % 2) * BLK for b in range(NB)]


def _build_program(debug_taps=False, add_on_pool=0,
                   relu2_on_act=True, a0_eng='act',
                   ACT_BUFS=2):
    nc = bacc.Bacc("TRN2", target_bir_lowering=False, debug=False,
                   num_devices=N_CORES)

    xk = nc.dram_tensor("x", [RPC, F], F32, kind="ExternalInput").ap()
    # weights pre-transposed/casted on host: lhsT tiles [K=128, M=128] bf16
    w0d = nc.dram_tensor("w0", [2, 128, 128], BF16, kind="ExternalInput").ap()
    w1d = nc.dram_tensor("w1", [2, 2, 128, 128], BF16,
                         kind="ExternalInput").ap()
    w2d = nc.dram_tensor("w2", [2, 128, 128], BF16, kind="ExternalInput").ap()
    # bias rows (already +1-folded), ones row: [K=1, 128/512] bf16
    bvd = nc.dram_tensor("bv", [5, 128], BF16, kind="ExternalInput").ap()
    oned = nc.dram_tensor("one", [1, BLK], BF16, kind="ExternalInput").ap()
    cntd = nc.dram_tensor("cnt", [128, NB * JPB], F32,
                          kind="ExternalInput").ap()
    out = nc.dram_tensor("o", [RPC, F], F32, kind="ExternalOutput").ap()
    taps = {}
    if debug_taps:
        for nm, sh in [("t_in", [128, BLK]), ("a0", [128, BLK]),
                       ("p0", [128, 2 * BLK]), ("e0", [128, 2 * BLK]),
                       ("h0", [128, 2 * BLK]), ("p1", [128, 2 * BLK]),
                       ("h1", [128, 2 * BLK]), ("p2", [128, BLK]),
                       ("h2", [128, BLK]), ("t_out", [128, BLK])]:
            taps[nm] = nc.dram_tensor(f"d_{nm}", sh, F32,
                                      kind="ExternalOutput").ap()

    ADD_ENG0 = nc.gpsimd if add_on_pool >= 1 else nc.vector
    ADD_ENG1 = nc.gpsimd if add_on_pool >= 2 else nc.vector
    ADD_ENG2 = nc.gpsimd if add_on_pool >= 3 else nc.vector
    with tile.TileContext(nc) as tc:
        with (
            tc.tile_pool(name="const", bufs=1) as cpool,
            tc.tile_pool(name="xin", bufs=3) as xpool,
            tc.tile_pool(name="act", bufs=ACT_BUFS) as apool,
            tc.tile_pool(name="yout", bufs=3) as ypool,
            tc.tile_pool(name="psA", bufs=1, space="PSUM") as ppoolA,
            tc.tile_pool(name="psB", bufs=1, space="PSUM") as ppoolB,
        ):
            identf = cpool.tile([128, 128], F32, tag="identf")
            make_identity(nc, identf[:])
            identb = cpool.tile([128, 128], BF16, tag="identb")
            nc.vector.tensor_copy(out=identb[:], in_=identf[:])

            w0t = cpool.tile([128, 2 * 128], BF16, tag="w0")
            for m in range(2):
                nc.sync.dma_start(out=w0t[:, m * 128:(m + 1) * 128],
                                  in_=w0d[m])
            w1t = cpool.tile([128, 4 * 128], BF16, tag="w1")
            for k in range(2):
                for m in range(2):
                    c = (k * 2 + m) * 128
                    nc.sync.dma_start(out=w1t[:, c:c + 128], in_=w1d[k, m])
            w2t = cpool.tile([128, 2 * 128], BF16, tag="w2")
            for m in range(2):
                nc.sync.dma_start(out=w2t[:, m * 128:(m + 1) * 128],
                                  in_=w2d[m])

            def w0_(m):
                return w0t[:, m * 128:(m + 1) * 128]

            def w1_(k, m):
                return w1t[:, (k * 2 + m) * 128:(k * 2 + m + 1) * 128]

            def w2_(k):
                return w2t[:, k * 128:(k + 1) * 128]

            # bias rows as separate [1,128] tiles (lhsT base partition
            # must be 0)
            bvt = []
            for i in range(5):
                bt = cpool.tile([1, 128], BF16, tag=f"bv{i}", name=f"bv{i}")
                nc.sync.dma_start(out=bt[:], in_=bvd[i:i + 1, :])
                bvt.append(bt)
            ones = cpool.tile([1, BLK], BF16, tag="ones")
            nc.sync.dma_start(out=ones[:], in_=oned)

            cnt = cpool.tile([128, NB * JPB], F32, tag="cnt")
            nc.sync.dma_start(out=cnt[:], in_=cntd)
            neg1 = cpool.tile([128, 1], F32, tag="neg1")
            nc.vector.memset(neg1[:], -1.0)

            ls = _load_starts()
            xin = {}
            for li in range(min(3, NL)):
                xin[li] = xpool.tile([128, LOAD], F32, tag=f"x{li % 3}",
                                     name=f"xin{li}")
                s = ls[li]
                nc.sync.dma_start(
                    out=xin[li][:].rearrange("p (a f) -> p a f", f=F),
                    in_=xk[s:s + LOAD, :].rearrange("(a p) f -> p a f",
                                                    p=128))

            bs = _block_starts()
            for b in range(NB):
                li = b // 2
                xt = xin[li]
                xoff = (b % 2) * BLK      # column offset into the load tile

                t_in = ppoolA.tile([128, BLK], F32, tag="t_in")
                for j in range(JPB):
                    nc.tensor.transpose(
                        out=t_in[:, j * 128:(j + 1) * 128],
                        in_=xt[:, xoff + j * 128:xoff + (j + 1) * 128],
                        identity=identf[:])
                a0 = apool.tile([128, BLK], BF16, tag="a0")
                if a0_eng == 'act':
                    nc.scalar.activation(a0[:], t_in[:], AF.Copy)
                elif a0_eng == 'pool':
                    nc.gpsimd.tensor_copy(out=a0[:], in_=t_in[:])
                else:
                    nc.vector.tensor_copy(out=a0[:], in_=t_in[:])

                # ---- L0: p0[:, m*BLK:...] = a0 @ W0_m + (b0_m + 1) ----
                p0 = ppoolA.tile([128, 2 * BLK], F32, tag="p0")
                for m in range(2):
                    sl = p0[:, m * BLK:(m + 1) * BLK]
                    nc.tensor.matmul(out=sl, lhsT=w0_(m), rhs=a0[:],
                                     start=True, stop=False)
                    nc.tensor.matmul(out=sl, lhsT=bvt[m][:],
                                     rhs=ones[:], start=False, stop=True)
                e0 = apool.tile([128, 2 * BLK], BF16, tag="e0")
                nc.scalar.activation(e0[:], p0[:], AF.Exp, bias=neg1[:, 0:1])
                r0 = apool.tile([128, 2 * BLK], BF16, tag="r0")
                nc.vector.tensor_scalar(out=r0[:], in0=p0[:], scalar1=1.0,
                                        scalar2=0.0, op0=OP.subtract,
                                        op1=OP.max)
                w0_t = apool.tile([128, 2 * BLK], BF16, tag="w0t")
                nc.vector.tensor_scalar(out=w0_t[:], in0=e0[:], scalar1=1.0,
                                        scalar2=None, op0=OP.min)
                h0 = apool.tile([128, 2 * BLK], BF16, tag="h0")
                ADD_ENG0.tensor_tensor(out=h0[:], in0=r0[:], in1=w0_t[:],
                                       op=OP.add)

                # ---- L1 ----
                p1 = ppoolB.tile([128, 2 * BLK], F32, tag="p1")
                for m in range(2):
                    sl = p1[:, m * BLK:(m + 1) * BLK]
                    nc.tensor.matmul(out=sl, lhsT=w1_(0, m),
                                     rhs=h0[:, 0:BLK], start=True, stop=False)
                    nc.tensor.matmul(out=sl, lhsT=w1_(1, m),
                                     rhs=h0[:, BLK:2 * BLK],
                                     start=False, stop=False)
                    nc.tensor.matmul(out=sl, lhsT=bvt[2 + m][:],
                                     rhs=ones[:], start=False, stop=True)
                e1 = apool.tile([128, 2 * BLK], BF16, tag="e1")
                nc.scalar.activation(e1[:], p1[:], AF.Exp, bias=neg1[:, 0:1])
                r1 = apool.tile([128, 2 * BLK], BF16, tag="r1")
                nc.vector.tensor_scalar(out=r1[:], in0=p1[:], scalar1=1.0,
                                        scalar2=0.0, op0=OP.subtract,
                                        op1=OP.max)
                w1_t = apool.tile([128, 2 * BLK], BF16, tag="w1t")
                nc.vector.tensor_scalar(out=w1_t[:], in0=e1[:], scalar1=1.0,
                                        scalar2=None, op0=OP.min)
                h1 = apool.tile([128, 2 * BLK], BF16, tag="h1")
                ADD_ENG1.tensor_tensor(out=h1[:], in0=r1[:], in1=w1_t[:],
                                       op=OP.add)

                # ---- L2 ----
                p2 = ppoolB.tile([128, BLK], F32, tag="p2")
                nc.tensor.matmul(out=p2[:], lhsT=w2_(0), rhs=h1[:, 0:BLK],
                                 start=True, stop=False)
                nc.tensor.matmul(out=p2[:], lhsT=w2_(1),
                                 rhs=h1[:, BLK:2 * BLK],
                                 start=False, stop=False)
                nc.tensor.matmul(out=p2[:], lhsT=bvt[4][:], rhs=ones[:],
                                 start=False, stop=True)
                e2 = apool.tile([128, BLK], BF16, tag="e2")
                nc.scalar.activation(e2[:], p2[:], AF.Exp, bias=neg1[:, 0:1])
                r2 = apool.tile([128, BLK], BF16, tag="r2")
                if relu2_on_act:
                    nc.scalar.activation(r2[:], p2[:], AF.Relu,
                                         bias=neg1[:, 0:1])
                else:
                    nc.vector.tensor_scalar(out=r2[:], in0=p2[:],
                                            scalar1=1.0, scalar2=0.0,
                                            op0=OP.subtract, op1=OP.max)
                w2_t = apool.tile([128, BLK], BF16, tag="w2t")
                nc.vector.tensor_scalar(out=w2_t[:], in0=e2[:], scalar1=1.0,
                                        scalar2=None, op0=OP.min)
                h2 = apool.tile([128, BLK], F32, tag="h2")
                ADD_ENG2.tensor_tensor(out=h2[:], in0=r2[:], in1=w2_t[:],
                                       op=OP.add)

                # ---- back to row-major; y = x + cnt*h' - cnt ----
                t_out = ppoolA.tile([128, BLK], F32, tag="t_out")
                for j in range(JPB):
                    nc.tensor.transpose(
                        out=t_out[:, j * 128:(j + 1) * 128],
                        in_=h2[:, j * 128:(j + 1) * 128],
                        identity=identf[:])
                y = ypool.tile([128, BLK], F32, tag="y")
                for j in range(JPB):
                    ca = cnt[:, b * JPB + j:b * JPB + j + 1]
                    yt = apool.tile([128, 128], F32, tag=f"yt{j}")
                    nc.vector.tensor_scalar(
                        out=yt[:], in0=t_out[:, j * 128:(j + 1) * 128],
                        scalar1=ca, scalar2=ca, op0=OP.mult, op1=OP.subtract)
                    nc.vector.tensor_tensor(
                        out=y[:, j * 128:(j + 1) * 128], in0=yt[:],
                        in1=xt[:, xoff + j * 128:xoff + (j + 1) * 128],
                        op=OP.add)
                if debug_taps and b == 0:
                    for nm, t in [("t_in", t_in), ("a0", a0), ("p0", p0),
                                  ("e0", e0), ("h0", h0), ("p1", p1),
                                  ("h1", h1), ("p2", p2), ("h2", h2),
                                  ("t_out", t_out)]:
                        tmp = apool.tile(list(t.shape), F32, tag=f"dt_{nm}",
                                         name=f"dt_{nm}")
                        nc.vector.tensor_copy(out=tmp[:], in_=t[:])
                        nc.sync.dma_start(out=taps[nm], in_=tmp[:])
                s = bs[b]
                nc.scalar.dma_start(
                    out=out[s:s + BLK, :].rearrange("(a p) f -> p a f",
                                                    p=128),
                    in_=y[:].rearrange("p (a f) -> p a f", f=F))

                # prefetch next load
                nli = li + 3
                if b % 2 == 1 and nli < NL:
                    xin[nli] = xpool.tile([128, LOAD], F32,
                                          tag=f"x{nli % 3}",
                                          name=f"xin{nli}")
                    nc.sync.dma_start(
                        out=xin[nli][:].rearrange("p (a f) -> p a f", f=F),
                        in_=xk[ls[nli]:ls[nli] + LOAD, :].rearrange(
                            "(a p) f -> p a f", p=128))
    return nc


def _prep_host(sel_idx, W0, b0, W1, b1, W2, b2):
    bf = mybir.dt.np(BF16)
    counts = np.bincount(sel_idx.reshape(-1), minlength=N).astype(np.float32)

    # fold elu's "-1" into the next layer's bias (inputs to L1/L2 are elu+1),
    # then fold the "+1" of min(exp(z), z+1) into the PSUM bias row.
    b0f = b0 + 1.0
    b1f = b1 - W1.sum(axis=0) + 1.0
    b2f = b2 - W2.sum(axis=0) + 1.0
    bv = np.stack([b0f[0:128], b0f[128:256], b1f[0:128], b1f[128:256],
                   b2f]).astype(bf)
    one = np.ones((1, BLK), dtype=bf)

    w0 = np.stack([W0[:, :128], W0[:, 128:]]).astype(bf)
    w1 = np.stack([
        np.stack([W1[0:128, 0:128], W1[0:128, 128:256]]),
        np.stack([W1[128:256, 0:128], W1[128:256, 128:256]]),
    ]).astype(bf)
    w2 = np.stack([W2[0:128, :], W2[128:256, :]]).astype(bf)

    bs = _block_starts()
    cnt_maps = []
    for k in range(N_CORES):
        ck = counts[k * RPC:(k + 1) * RPC]
        cm = np.empty((128, NB * JPB), np.float32)
        for b in range(NB):
            for j in range(JPB):
                s = bs[b] + j * 128
                cm[:, b * JPB + j] = ck[s:s + 128]
        cnt_maps.append(np.ascontiguousarray(cm))
    return w0, w1, w2, bv, one, cnt_maps


def kernel(x, sel_idx, W0, b0, W1, b1, W2, b2):
    x = np.ascontiguousarray(np.asarray(x, dtype=np.float32))
    sel_idx = np.asarray(sel_idx, dtype=np.int32)
    W0, W1, W2 = [np.ascontiguousarray(np.asarray(w, dtype=np.float32))
                  for w in (W0, W1, W2)]
    b0, b1, b2 = [np.asarray(b, dtype=np.float32) for b in (b0, b1, b2)]

    w0, w1, w2, bv, one, cnt_maps = _prep_host(sel_idx, W0, b0, W1, b1,
                                               W2, b2)
    nc = _build_program(add_on_pool=2)
    nc.compile()

    in_maps = []
    for k in range(N_CORES):
        in_maps.append({
            "x": x[k * RPC:(k + 1) * RPC],
            "w0": w0, "w1": w1, "w2": w2, "bv": bv, "one": one,
            "cnt": cnt_maps[k],
        })
    global LAST_RUN_RESULTS, LAST_NC, LAST_IN_MAPS
    LAST_NC, LAST_IN_MAPS = nc, in_maps
    res = run_bass_kernel_spmd(nc, in_maps, core_ids=list(range(N_CORES)))
    LAST_RUN_RESULTS = res

    out = np.empty_like(x)
    for k in range(N_CORES):
        out[k * RPC:(k + 1) * RPC] = res.results[k]["o"]
    return out
